# revision 1
# baseline (speedup 1.0000x reference)
"""Trainium2 Bass kernel for a dense transformer block (B=4,S=1024,D=1024,F=4096,H=16).

Sharding: 8 cores = (batch b in 0..3) x (seq half). Pure SPMD, no collectives:
the host rotates each core's tokens so its 512 query rows are always rows
0..511 of the rotated sequence; K/V cover the full (rotated) sequence.

Layout strategy per core:
  - LN1 token-major (bn_stats), output bf16, PE-transposed to feature-major y1T.
  - q^T/k^T feature-major via matmul(lhsT=W chunk, rhs=y1T);
    V token-major via matmul(lhsT=y1T chunk, rhs=W) with a fused ones column
    (V_aug [tok, 16, 65]) so the softmax row-sum rides the ctx matmul.
  - scores computed key-major: s^T[k,q] = matmul(lhsT=k^T_h, rhs=q^T_h), exp on
    ScalarE (per-(b,h) bias folded into the activation bias), multiplicative
    {0,1} mask on VectorE, ctx^T + rowsum = matmul(lhsT=V_aug, rhs=E).
  - normalization: recip(rowsum) then fp32r ones-matmul partition-broadcast.
  - Wo/FFN2 token-major outputs (lhsT=activation chunk, rhs=W chunk).
  - LN gains/biases and the q 1/sqrt(d) scale are folded into weights on host.
All matmuls run in bf16 with fp32 PSUM accumulation.
"""

import numpy as np
import ml_dtypes

import concourse.bass as bass
import concourse.mybir as mybir
import concourse.tile as tile
from concourse import bacc
from concourse.bass_utils import run_bass_kernel_spmd

F32 = mybir.dt.float32
F32R = mybir.dt.float32r
BF16 = mybir.dt.bfloat16
BF = ml_dtypes.bfloat16

B, S, D, F, H = 4, 1024, 1024, 4096, 16
d = D // H          # 64
P = 128             # partitions
SQ = 512            # queries per core
EPS = 1e-5
NT = S // P         # 8 token tiles (full seq)
NQ = SQ // P        # 4 query tiles
NF = D // P         # 8 feature chunks
NF1 = F // P        # 32 ffn chunks

AX = mybir.AxisListType
ALU = mybir.AluOpType
ACTF = mybir.ActivationFunctionType


def _T(pool, shape, dtype, tag):
    return pool.tile(shape, dtype, name=tag, tag=tag)


def _pbcast(ap, p):
    """Partition-broadcast a [1, N] DRAM AP to [p, N]."""
    return bass.AP(tensor=ap.tensor, offset=ap.offset, ap=[[0, p]] + list(ap.ap[1:]))


def _build_program(FL, reps=1):
    nc = bacc.Bacc("TRN2", target_bir_lowering=False, debug=False)

    t = {}
    t["x"] = nc.dram_tensor("x", [S, D], F32, kind="ExternalInput").ap()
    t["maskT"] = nc.dram_tensor("maskT", [S, SQ], BF16, kind="ExternalInput").ap()
    t["bias"] = nc.dram_tensor("bias", [1, H], F32, kind="ExternalInput").ap()
    t["wq"] = nc.dram_tensor("wq", [NF, P, D], BF16, kind="ExternalInput").ap()
    t["wk"] = nc.dram_tensor("wk", [NF, P, D], BF16, kind="ExternalInput").ap()
    t["wv"] = nc.dram_tensor("wv", [D, D], BF16, kind="ExternalInput").ap()
    t["wo"] = nc.dram_tensor("wo", [2 * NF, P, SQ], BF16, kind="ExternalInput").ap()
    t["w1"] = nc.dram_tensor("w1", [NF1, P, D], BF16, kind="ExternalInput").ap()
    t["w2"] = nc.dram_tensor("w2", [2 * NF1, P, SQ], BF16, kind="ExternalInput").ap()
    t["bq"] = nc.dram_tensor("bq", [1, D], BF16, kind="ExternalInput").ap()
    t["bk"] = nc.dram_tensor("bk", [1, D], BF16, kind="ExternalInput").ap()
    t["bv"] = nc.dram_tensor("bv", [1, D], BF16, kind="ExternalInput").ap()
    t["bo"] = nc.dram_tensor("bo", [1, D], F32, kind="ExternalInput").ap()
    t["b1"] = nc.dram_tensor("b1", [1, F], BF16, kind="ExternalInput").ap()
    t["b2"] = nc.dram_tensor("b2", [1, D], BF16, kind="ExternalInput").ap()
    t["ident"] = nc.dram_tensor("ident", [P, P], BF16, kind="ExternalInput").ap()
    t["out"] = nc.dram_tensor("out", [SQ, D], F32, kind="ExternalOutput").ap()

    with tile.TileContext(nc) as tc:
        for rep in range(reps):
            _trace(nc, tc, t, FL, pfx=f"r{rep}_" if reps > 1 else "")
    nc.compile()
    return nc


def _layernorm_tile(nc, pool, x_ap, out_ap, epst):
    """out = (x - mean(x)) * rsqrt(var(x) + eps) along the 1024-wide free dim."""
    st = _T(pool, [P, 2, 6], F32, "st")
    xr = x_ap.rearrange("p (a b) -> p a b", b=512)
    for sg in range(2):
        nc.vector.bn_stats(out=st[:, sg, :], in_=xr[:, sg, :])
    mv = _T(pool, [P, 2], F32, "mv")
    nc.vector.bn_aggr(out=mv[:], in_=st[:])
    rs = _T(pool, [P, 1], F32, "rs")
    nc.scalar.activation(out=rs[:], in_=mv[:, 1:2], func=ACTF.Sqrt,
                         bias=epst[:], scale=1.0)
    nc.vector.reciprocal(out=rs[:], in_=rs[:])
    nc.vector.tensor_scalar(out=out_ap, in0=x_ap, scalar1=mv[:, 0:1],
                            scalar2=rs[:], op0=ALU.subtract, op1=ALU.mult)


def _trace(nc, tc, t, FL, pfx=""):
    x3 = t["x"].rearrange("(n p) c -> n p c", p=P)
    mT3 = t["maskT"].rearrange("(n p) c -> n p c", p=P)
    out3 = t["out"].rearrange("(n p) c -> n p c", p=P)

    with (
        tc.tile_pool(name=pfx + "const", bufs=1) as const_p,
        tc.tile_pool(name=pfx + "small", bufs=3) as small_p,
        tc.tile_pool(name=pfx + "xres", bufs=NQ) as xres_p,
        tc.tile_pool(name=pfx + "ctxT", bufs=NF) as ctxT_p,
        tc.tile_pool(name=pfx + "x2", bufs=NQ) as x2_p,
        tc.tile_pool(name=pfx + "pt", bufs=2, space=bass.MemorySpace.PSUM) as ps_tp,
    ):
        # ---- constants ----
        ident = _T(const_p, [P, P], BF16, "ident")
        nc.sync.dma_start(out=ident[:], in_=t["ident"])
        if any(FL[k] for k in ("bq", "bk", "bv", "b1", "b2")):
            ones = _T(const_p, [1, SQ], BF16, "ones")
            nc.vector.memset(ones[:], 1.0)
        epst = _T(const_p, [P, 1], F32, "epst")
        nc.vector.memset(epst[:], EPS)
        if FL["bo"]:
            bo_b = _T(const_p, [P, D], F32, "bo_b")
            nc.sync.dma_start(out=bo_b[:], in_=_pbcast(t["bo"], P))
        brow = {}
        for nm in ("bq", "bk", "bv", "b1", "b2"):
            if not FL[nm]:
                continue
            sz = F if nm == "b1" else D
            brow[nm] = _T(const_p, [1, sz], BF16, f"brow_{nm}")
            nc.sync.dma_start(out=brow[nm][:], in_=t[nm])

        xres = [_T(xres_p, [P, D], F32, "xres") for _ in range(NQ)]
        ctxT = [_T(ctxT_p, [P, SQ], BF16, "ctxT") for _ in range(NF)]
        x2 = [_T(x2_p, [P, D], F32, "x2") for _ in range(NQ)]

        # ================= phase 1: LN1, y1T, QKV, attention =================
        with (
            tc.tile_pool(name=pfx + "xs", bufs=2) as xs_p,
            tc.tile_pool(name=pfx + "y1", bufs=2) as y1_p,
            tc.tile_pool(name=pfx + "y1T", bufs=NF) as y1T_p,
            tc.tile_pool(name=pfx + "wqk", bufs=3) as wqk_p,
            tc.tile_pool(name=pfx + "wv", bufs=NF) as wv_p,
            tc.tile_pool(name=pfx + "kT", bufs=NF) as kT_p,
            tc.tile_pool(name=pfx + "qT", bufs=NF) as qT_p,
            tc.tile_pool(name=pfx + "mk", bufs=NT) as mk_p,
            tc.tile_pool(name=pfx + "es", bufs=20) as e_p,
            tc.tile_pool(name=pfx + "pm", bufs=4, space=bass.MemorySpace.PSUM) as ps_mm,
            tc.tile_pool(name=pfx + "pc", bufs=2, space=bass.MemorySpace.PSUM) as ps_ctx,
        ):
            y1T = [_T(y1T_p, [P, S], BF16, "y1T") for _ in range(NF)]

            # LN1 over full (rotated) sequence + transpose to feature-major
            for tt in range(NT):
                if tt < NQ:
                    xt = xres[tt]
                else:
                    xt = _T(xs_p, [P, D], F32, "xs")
                nc.sync.dma_start(out=xt[:], in_=x3[tt])
                yt = _T(y1_p, [P, D], BF16, "y1")
                _layernorm_tile(nc, small_p, xt[:], yt[:], epst)
                for fc in range(NF):
                    pt = _T(ps_tp, [P, P], BF16, "pt")
                    nc.tensor.transpose(pt[:], yt[:, fc * P:(fc + 1) * P], ident[:])
                    nc.scalar.copy(out=y1T[fc][:, tt * P:(tt + 1) * P], in_=pt[:])

            # mask tiles (key-major, bf16 {0,1})
            mT = [_T(mk_p, [P, SQ], BF16, "mk") for _ in range(NT)]
            for kt in range(NT):
                nc.sync.dma_start(out=mT[kt][:], in_=mT3[kt])

            # q^T: [f_out 128, q 512] per chunk (only own 512 queries)
            qT = [_T(qT_p, [P, SQ], BF16, "qT") for _ in range(NF)]
            for fo in range(NF):
                wt = _T(wqk_p, [P, D], BF16, "wqk")
                nc.scalar.dma_start(out=wt[:], in_=t["wq"][fo])
                ps = _T(ps_mm, [P, SQ], F32, "pm")
                ps = ps[:]
                for fi in range(NF):
                    nc.tensor.matmul(ps, wt[:, fi * P:(fi + 1) * P],
                                     y1T[fi][:, 0:SQ],
                                     start=(fi == 0),
                                     stop=(not FL["bq"] and fi == NF - 1))
                if FL["bq"]:
                    nc.tensor.matmul(ps, brow["bq"][:, fo * P:(fo + 1) * P],
                                     ones[:], start=False, stop=True)
                nc.vector.tensor_copy(out=qT[fo][:], in_=ps)

            # k^T: [f_out 128, tok 1024] per chunk (full sequence)
            kT = [_T(kT_p, [P, S], BF16, "kT") for _ in range(NF)]

            def kT_chunk(fo):
                wt = _T(wqk_p, [P, D], BF16, "wqk")
                nc.scalar.dma_start(out=wt[:], in_=t["wk"][fo])
                for th in range(2):
                    tsl = slice(th * SQ, (th + 1) * SQ)
                    ps = _T(ps_mm, [P, SQ], F32, "pm")
                    for fi in range(NF):
                        nc.tensor.matmul(ps[:], wt[:, fi * P:(fi + 1) * P],
                                         y1T[fi][:, tsl],
                                         start=(fi == 0),
                                         stop=(not FL["bk"] and fi == NF - 1))
                    if FL["bk"]:
                        nc.tensor.matmul(ps[:], brow["bk"][:, fo * P:(fo + 1) * P],
                                         ones[:], start=False, stop=True)
                    nc.vector.tensor_copy(out=kT[fo][:, tsl], in_=ps[:])

            def score_kt(h, kt):
                """scores^T -> exp -> mask for one (head, key-tile); returns E.
                Even/odd heads sit at PE row groups 0-63/64-127 and can run
                concurrently when adjacent in the PE stream."""
                fc, po = h // 2, (h % 2) * d
                ps_s = _T(ps_mm, [P, SQ], F32, "pm")
                nc.tensor.matmul(ps_s[:],
                                 kT[fc][po:po + d, kt * P:(kt + 1) * P],
                                 qT[fc][po:po + d, :],
                                 start=True, stop=True)
                e = _T(e_p, [P, SQ], BF16, "e")
                nc.scalar.activation(out=e[:], in_=ps_s[:], func=ACTF.Exp)
                nc.vector.tensor_mul(e[:], e[:], mT[kt][:])
                return e

            def ctx_mm(pcs, h, kt, e):
                nc.tensor.matmul(pcs[:], V[kt][:, h, :], e[:],
                                 start=(kt == 0), stop=(kt == NT - 1),
                                 skip_group_check=True)

            def ctx_norm(h, pcs):
                fco, po = h // 2, (h % 2) * d
                rb = _T(small_p, [d, SQ], F32, "rb")
                nc.vector.reciprocal(rb[:], pcs[d:2 * d, :])
                nc.vector.tensor_mul(ctxT[fco][po:po + d, :], pcs[0:d, :], rb[:])

            # pair 0 first: its exp stream starts while V is produced
            kT_chunk(0)
            e01 = {0: [], 1: []}
            for kt in range(NT):
                for h in (0, 1):
                    e01[h].append(score_kt(h, kt))

            # V token-major, heads interleaved d-cols then d ones-cols
            wv_t = [_T(wv_p, [P, D], BF16, "wv") for _ in range(NF)]
            for fi in range(NF):
                nc.scalar.dma_start(out=wv_t[fi][:],
                                  in_=t["wv"].rearrange("(n p) c -> n p c", p=P)[fi])
            V = [_T(kT_p, [P, H, 2 * d], BF16, "V") for _ in range(NT)]
            for kt in range(NT):
                nc.gpsimd.memset(V[kt][:, :, d:], 1.0)
                for fh in range(2):
                    fsl = slice(fh * SQ, (fh + 1) * SQ)
                    ps = _T(ps_mm, [P, SQ], F32, "pm")
                    for fi in range(NF):
                        nc.tensor.matmul(ps[:],
                                         y1T[fi][:, kt * P:(kt + 1) * P],
                                         wv_t[fi][:, fsl],
                                         start=(fi == 0),
                                         stop=(not FL["bv"] and fi == NF - 1))
                    if FL["bv"]:
                        nc.tensor.matmul(ps[:], ones[:, 0:P], brow["bv"][:, fsl],
                                         start=False, stop=True)
                    nc.vector.tensor_copy(
                        out=V[kt][:, fh * (H // 2):(fh + 1) * (H // 2), 0:d],
                        in_=ps[:].rearrange("p (a b) -> p a b", b=d))

            pcs0 = {h: _T(ps_ctx, [P, SQ], F32, "pc") for h in (0, 1)}
            for kt in range(NT):
                for h in (0, 1):
                    ctx_mm(pcs0[h], h, kt, e01[h][kt])
            for h in (0, 1):
                ctx_norm(h, pcs0[h])
            del e01, pcs0

            LAG = 3
            for fc in range(1, NF):
                kT_chunk(fc)
                pair = (2 * fc, 2 * fc + 1)
                es = {h: [] for h in pair}
                pcs = {h: _T(ps_ctx, [P, SQ], F32, "pc") for h in pair}
                for kt in range(NT):
                    for h in pair:
                        es[h].append(score_kt(h, kt))
                    if kt >= LAG:
                        for h in pair:
                            ctx_mm(pcs[h], h, kt - LAG, es[h][kt - LAG])
                for kt in range(NT - LAG, NT):
                    for h in pair:
                        ctx_mm(pcs[h], h, kt, es[h][kt])
                for h in pair:
                    ctx_norm(h, pcs[h])

        # ================= phase 2: Wo, LN2, FFN =================
        with (
            tc.tile_pool(name=pfx + "wh", bufs=10) as wh_p,
            tc.tile_pool(name=pfx + "wo", bufs=2 * NF) as wo_p,
            tc.tile_pool(name=pfx + "w1s", bufs=2) as w1_p,
            tc.tile_pool(name=pfx + "hT", bufs=NF1) as hT_p,
            tc.tile_pool(name=pfx + "y2", bufs=2) as y2_p,
            tc.tile_pool(name=pfx + "y2T", bufs=NF) as y2T_p,
            tc.tile_pool(name=pfx + "xo", bufs=NQ) as xo_p,
            tc.tile_pool(name=pfx + "pm2", bufs=2, space=bass.MemorySpace.PSUM) as ps_mm,
            tc.tile_pool(name=pfx + "p4", bufs=4, space=bass.MemorySpace.PSUM) as ps_4,
        ):
            # Wo: resident weights, qt-outer so LN2/y2T pipeline per q-tile
            wo_t = [_T(wo_p, [P, SQ], BF16, "woW") for _ in range(2 * NF)]
            for i in range(2 * NF):
                nc.scalar.dma_start(out=wo_t[i][:], in_=t["wo"][i])
            y2T = [_T(y2T_p, [P, SQ], BF16, "y2T") for _ in range(NF)]
            for qt in range(NQ):
                for dh in range(2):
                    dsl = slice(dh * SQ, (dh + 1) * SQ)
                    ps = _T(ps_mm, [P, SQ], F32, "pm")
                    for fi in range(NF):
                        nc.tensor.matmul(ps[:],
                                         ctxT[fi][:, qt * P:(qt + 1) * P],
                                         wo_t[dh * NF + fi][:],
                                         start=(fi == 0), stop=(fi == NF - 1))
                    nc.vector.tensor_add(x2[qt][:, dsl], ps[:], xres[qt][:, dsl])
                    if FL["bo"]:
                        nc.vector.tensor_add(x2[qt][:, dsl], x2[qt][:, dsl],
                                             bo_b[:, dsl])
                yt = _T(y2_p, [P, D], BF16, "y2")
                _layernorm_tile(nc, small_p, x2[qt][:], yt[:], epst)
                for fc in range(NF):
                    pt = _T(ps_tp, [P, P], BF16, "pt")
                    nc.tensor.transpose(pt[:], yt[:, fc * P:(fc + 1) * P], ident[:])
                    nc.scalar.copy(out=y2T[fc][:, qt * P:(qt + 1) * P], in_=pt[:])

            # FFN1: h^T[f1 128, q 512] = gelu(W1' y2 + b1')
            hT = [_T(hT_p, [P, SQ], BF16, "hT") for _ in range(NF1)]
            for f1 in range(NF1):
                wt = _T(w1_p, [P, D], BF16, "w1s")
                nc.scalar.dma_start(out=wt[:], in_=t["w1"][f1])
                ps = _T(ps_mm, [P, SQ], F32, "pm")
                for fi in range(NF):
                    nc.tensor.matmul(ps[:], wt[:, fi * P:(fi + 1) * P], y2T[fi][:],
                                     start=(fi == 0),
                                     stop=(not FL["b1"] and fi == NF - 1))
                if FL["b1"]:
                    nc.tensor.matmul(ps[:], brow["b1"][:, f1 * P:(f1 + 1) * P],
                                     ones[:], start=False, stop=True)
                nc.scalar.activation(out=hT[f1][:], in_=ps[:], func=ACTF.Gelu)

            # FFN2 + residual: out = x2 + h @ W2 + b2
            xout = [_T(xo_p, [P, D], F32, "xo") for _ in range(NQ)]
            for dh in range(2):
                dsl = slice(dh * SQ, (dh + 1) * SQ)
                ps4 = [_T(ps_4, [P, SQ], F32, "p4") for _ in range(NQ)]
                for f1 in range(NF1):
                    wt = _T(wh_p, [P, SQ], BF16, "wh")
                    nc.scalar.dma_start(out=wt[:], in_=t["w2"][dh * NF1 + f1])
                    for qt in range(NQ):
                        nc.tensor.matmul(ps4[qt][:],
                                         hT[f1][:, qt * P:(qt + 1) * P], wt[:],
                                         start=(f1 == 0),
                                         stop=(not FL["b2"] and f1 == NF1 - 1))
                for qt in range(NQ):
                    if FL["b2"]:
                        nc.tensor.matmul(ps4[qt][:], ones[:, 0:P],
                                         brow["b2"][:, dsl],
                                         start=False, stop=True)
                    nc.vector.tensor_add(xout[qt][:, dsl], ps4[qt][:],
                                         x2[qt][:, dsl])
                    nc.sync.dma_start(out=out3[qt][:, dsl],
                                      in_=xout[qt][:, dsl])


_NC = {}
_ALL_FLAGS = ("bq", "bk", "bv", "bo", "b1", "b2")


def _get_nc(flags=None, reps=1):
    if flags is None:
        flags = {k: True for k in _ALL_FLAGS}
    key = (tuple(sorted(flags.items())), reps)
    if key not in _NC:
        _NC[key] = _build_program(dict(flags), reps=reps)
    return _NC[key]


def _prep_inputs(inputs):
    """Host-side folding + per-core shard maps."""
    x = np.asarray(inputs["x"], np.float32)
    attn_bias = np.asarray(inputs["attn_bias"], np.float32)
    mask = np.asarray(inputs["mask"], np.float32)
    g1 = np.asarray(inputs["ln1_g"], np.float32)
    b1n = np.asarray(inputs["ln1_b"], np.float32)
    g2 = np.asarray(inputs["ln2_g"], np.float32)
    b2n = np.asarray(inputs["ln2_b"], np.float32)
    Wq = np.asarray(inputs["Wq"], np.float32); bq = np.asarray(inputs["bq"], np.float32)
    Wk = np.asarray(inputs["Wk"], np.float32); bk = np.asarray(inputs["bk"], np.float32)
    Wv = np.asarray(inputs["Wv"], np.float32); bv = np.asarray(inputs["bv"], np.float32)
    Wo = np.asarray(inputs["Wo"], np.float32); bo = np.asarray(inputs["bo"], np.float32)
    W1 = np.asarray(inputs["W1"], np.float32); b1 = np.asarray(inputs["b1"], np.float32)
    W2 = np.asarray(inputs["W2"], np.float32); b2 = np.asarray(inputs["b2"], np.float32)

    scale = d ** -0.5
    # fold LN gains/biases (and q scale) into the projection weights
    Wq_e = (g1[:, None] * Wq) * scale
    bq_e = (b1n @ Wq + bq) * scale
    Wk_e = g1[:, None] * Wk
    bk_e = b1n @ Wk + bk
    Wv_e = g1[:, None] * Wv
    bv_e = b1n @ Wv + bv
    W1_e = g2[:, None] * W1
    b1_e = b2n @ W1 + b1

    def tile_fo(W):
        # [Din, Dout] -> [NF(fo), P, NF(fi)*P] with partition = f_in-within-chunk
        return np.ascontiguousarray(
            W.reshape(NF, P, NF, P).transpose(2, 1, 0, 3).reshape(NF, P, D)
        )

    wq_h = tile_fo(Wq_e).astype(BF)
    wk_h = tile_fo(Wk_e).astype(BF)
    wv_h = np.ascontiguousarray(Wv_e).astype(BF)
    # Wo -> [2*NF (dh,fi), P, SQ]
    wo_h = np.ascontiguousarray(
        Wo.reshape(NF, P, 2, SQ).transpose(2, 0, 1, 3).reshape(2 * NF, P, SQ)
    ).astype(BF)
    # W1 -> [NF1(f1), P(f_in within chunk), NF(fi)*P]
    w1_h = np.ascontiguousarray(
        W1_e.reshape(NF, P, NF1, P).transpose(2, 1, 0, 3).reshape(NF1, P, D)
    ).astype(BF)
    # W2 -> [2*NF1 (dh,f1), P, SQ]
    w2_h = np.ascontiguousarray(
        W2.reshape(NF1, P, 2, SQ).transpose(2, 0, 1, 3).reshape(2 * NF1, P, SQ)
    ).astype(BF)

    flags = {
        "bq": bool(np.any(bq_e)), "bk": bool(np.any(bk_e)),
        "bv": bool(np.any(bv_e)), "bo": bool(np.any(bo)),
        "b1": bool(np.any(b1_e)), "b2": bool(np.any(b2)),
    }
    shared = {
        "ident": np.eye(P, dtype=BF),
        "wq": wq_h, "wk": wk_h, "wv": wv_h, "wo": wo_h, "w1": w1_h, "w2": w2_h,
        "bq": bq_e.reshape(1, D).astype(BF),
        "bk": bk_e.reshape(1, D).astype(BF),
        "bv": bv_e.reshape(1, D).astype(BF),
        "bo": bo.reshape(1, D).astype(np.float32),
        "b1": b1_e.reshape(1, F).astype(BF),
        "b2": b2.reshape(1, D).astype(BF),
    }

    in_maps = []
    for c in range(8):
        b, hf = c // 2, c % 2
        x_rot = np.ascontiguousarray(np.roll(x[b], -SQ * hf, axis=0))
        mq = mask[b, 0, SQ * hf:SQ * (hf + 1), :]          # [512 q, 1024 k]
        mT = np.ascontiguousarray(np.roll(mq.T, -SQ * hf, axis=0)).astype(BF)
        bias_c = (attn_bias[b - 1] if b > 0 else np.zeros(H, np.float32))
        m = dict(shared)
        m["x"] = x_rot
        m["maskT"] = mT
        m["bias"] = bias_c.reshape(1, H).astype(np.float32)
        in_maps.append(m)
    return in_maps, flags


def run(inputs, trace=False, **kw):
    in_maps, flags = _prep_inputs(inputs)
    nc = _get_nc(flags)
    res = run_bass_kernel_spmd(nc, in_maps, core_ids=list(range(8)),
                               trace=trace, **kw)
    out = np.empty((B, S, D), np.float32)
    for c in range(8):
        b, hf = c // 2, c % 2
        out[b, SQ * hf:SQ * (hf + 1), :] = res.results[c]["out"]
    return out, res


def kernel(**inputs) -> np.ndarray:
    out, _ = run(inputs, trace=False)
    return out



# revision 15
# speedup vs baseline: 19.2294x; 19.2294x over previous
"""Trainium2 Bass kernel for a dense transformer block (B=4,S=1024,D=1024,F=4096,H=16).

Sharding: 8 cores = (batch b in 0..3) x (seq half). Pure SPMD, no collectives:
the host rotates each core's tokens so its 512 query rows are always rows
0..511 of the rotated sequence; K/V cover the full (rotated) sequence.

v2: fp8e4m3 DoubleRow matmuls (0.5 cyc/row) for the QKV projections and Wo.
  - y1 = LN1(x) transposed to a single interleaved fp8 tile y1T8 [128, 8, 1024]
    so each DoubleRow matmul contracts 256 features (2 chunks of 128).
  - weights are pre-scaled on host (Wq*128, Wk*16, Wv*16, Wo*16) to lift the
    0.02-scale entries out of fp8 subnormal range; the score exp absorbs
    1/(128*16) via its activation scale, the softmax ones-column is set to 16
    to absorb V's scale, and an ACT copy with scale=1/16 rescales Wo's output.
  - scores/ctx and the FFN stay bf16 (fp8 there costs too much accuracy).
Scheduling: x DMAs lead the SP queue (weights follow, FFN weights prefetched
through outer-scope stream pools), LN runs split 0-3 / 4-7 so qT and the kT
half-chunks start as soon as their token tiles are normalized, exps are
batched over kt-pairs ([128,1024] PSUM reads), mask-multiply + LN1 apply +
V-ones memsets + part of the copies/adds run on GpSimd(Pool).
The softmax row-sum rides the ctx matmul as 16.0-columns of V_aug; the
per-(b,h) additive attn_bias is constant over q and k so softmax cancels it.
"""

import numpy as np
import ml_dtypes

import concourse.bass as bass
import concourse.mybir as mybir
import concourse.tile as tile
from concourse import bacc
from concourse.bass_utils import run_bass_kernel_spmd

F32 = mybir.dt.float32
BF16 = mybir.dt.bfloat16
F8 = mybir.dt.float8e4
BF = ml_dtypes.bfloat16
E4 = ml_dtypes.float8_e4m3

B, S, D, F, H = 4, 1024, 1024, 4096, 16
d = D // H          # 64
P = 128             # partitions
SQ = 512            # queries per core
EPS = 1e-5
NT = S // P         # 8 token tiles (full seq)
NQ = SQ // P        # 4 query tiles
NF = D // P         # 8 feature chunks
NJ = NF // 2        # 4 fp8 contraction pairs
NF1 = F // P        # 32 ffn chunks

SCQ = 128.0         # host pre-scale on Wq (includes d^-0.5 fold)
SCK = 16.0          # host pre-scale on Wk
SCV = 16.0          # host pre-scale on Wv (absorbed by 16.0 ones-column)
SCO = 16.0          # host pre-scale on Wo (rescaled by ACT copy 1/16)

AX = mybir.AxisListType
ALU = mybir.AluOpType
ACTF = mybir.ActivationFunctionType
DR = mybir.MatmulPerfMode.DoubleRow


def _T(pool, shape, dtype, tag):
    return pool.tile(shape, dtype, name=tag, tag=tag)


def _pbcast(ap, p):
    """Partition-broadcast a [1, N] DRAM AP to [p, N]."""
    return bass.AP(tensor=ap.tensor, offset=ap.offset, ap=[[0, p]] + list(ap.ap[1:]))


def _build_program(FL, reps=1):
    nc = bacc.Bacc("TRN2", target_bir_lowering=False, debug=False)

    t = {}
    t["x"] = nc.dram_tensor("x", [SQ, D], F32, kind="ExternalInput").ap()
    t["xk"] = nc.dram_tensor("xk", [SQ, D], BF16, kind="ExternalInput").ap()
    t["maskT2"] = nc.dram_tensor("maskT2", [NT // 2, P, 2 * SQ], mybir.dt.uint8,
                                 kind="ExternalInput").ap()
    t["wq"] = nc.dram_tensor("wq", [NJ, P, 2, D], F8, kind="ExternalInput").ap()
    t["wk"] = nc.dram_tensor("wk", [NJ, P, 2, D], F8, kind="ExternalInput").ap()
    t["wv"] = nc.dram_tensor("wv", [NJ, P, 2, D], F8, kind="ExternalInput").ap()
    t["wo"] = nc.dram_tensor("wo", [2 * NJ, P, 2, SQ], F8, kind="ExternalInput").ap()
    t["w1"] = nc.dram_tensor("w1", [NF1, P, D], BF16, kind="ExternalInput").ap()
    t["w2"] = nc.dram_tensor("w2", [2 * NF1, P, SQ], BF16, kind="ExternalInput").ap()
    t["bq"] = nc.dram_tensor("bq", [1, D], BF16, kind="ExternalInput").ap()
    t["bk"] = nc.dram_tensor("bk", [1, D], BF16, kind="ExternalInput").ap()
    t["bv"] = nc.dram_tensor("bv", [1, D], BF16, kind="ExternalInput").ap()
    t["bo"] = nc.dram_tensor("bo", [1, D], F32, kind="ExternalInput").ap()
    t["b1"] = nc.dram_tensor("b1", [1, F], BF16, kind="ExternalInput").ap()
    t["b2"] = nc.dram_tensor("b2", [1, D], BF16, kind="ExternalInput").ap()
    t["ident"] = nc.dram_tensor("ident", [P, P], BF16, kind="ExternalInput").ap()
    t["out"] = nc.dram_tensor("out", [SQ, D], F32, kind="ExternalOutput").ap()

    with tile.TileContext(nc) as tc:
        for rep in range(reps):
            _trace(nc, tc, t, FL, pfx=f"r{rep}_" if reps > 1 else "")
    nc.compile()
    return nc


def _ln_stats(nc, pool, x_ap, epst):
    """Return [P,1] mean and rstd tiles for a [P, 1024] input."""
    st = _T(pool, [P, 2, 6], F32, "st")
    xr = x_ap.rearrange("p (a b) -> p a b", b=512)
    for sg in range(2):
        nc.vector.bn_stats(out=st[:, sg, :], in_=xr[:, sg, :])
    mv = _T(pool, [P, 2], F32, "mv")
    nc.vector.bn_aggr(out=mv[:], in_=st[:])
    rs = _T(pool, [P, 1], F32, "rs")
    nc.scalar.activation(out=rs[:], in_=mv[:, 1:2], func=ACTF.Sqrt,
                         bias=epst[:], scale=1.0)
    nc.vector.reciprocal(out=rs[:], in_=rs[:])
    return mv, rs


def _trace(nc, tc, t, FL, pfx=""):
    x3 = t["x"].rearrange("(n p) c -> n p c", p=P)
    xk3 = t["xk"].rearrange("(n p) c -> n p c", p=P)
    out3 = t["out"].rearrange("(n p) c -> n p c", p=P)

    with (
        tc.tile_pool(name=pfx + "const", bufs=1) as const_p,
        tc.tile_pool(name=pfx + "small", bufs=4) as small_p,
        tc.tile_pool(name=pfx + "xres", bufs=NQ) as xres_p,
        tc.tile_pool(name=pfx + "ctxT", bufs=1) as ctxT_p,
        tc.tile_pool(name=pfx + "w1s", bufs=6) as w1_p,
        tc.tile_pool(name=pfx + "w2s", bufs=12) as wh_p,
    ):
        # ---- constants ----
        ident = _T(const_p, [P, P], BF16, "ident")
        nc.gpsimd.dma_start(out=ident[:], in_=t["ident"])
        if any(FL[k] for k in ("bq", "bk", "bv", "b1", "b2")):
            ones = _T(const_p, [1, SQ], BF16, "ones")
            nc.vector.memset(ones[:], 1.0)
        epst = _T(const_p, [P, 1], F32, "epst")
        nc.vector.memset(epst[:], EPS)
        if FL["bo"]:
            bo_b = _T(const_p, [P, D], F32, "bo_b")
            nc.sync.dma_start(out=bo_b[:], in_=_pbcast(t["bo"], P))
        brow = {}
        for nm in ("bq", "bk", "bv", "b1", "b2"):
            if not FL[nm]:
                continue
            sz = F if nm == "b1" else D
            brow[nm] = _T(const_p, [1, sz], BF16, f"brow_{nm}")
            nc.sync.dma_start(out=brow[nm][:], in_=t[nm])

        xres = [_T(xres_p, [P, D], F32, "xres") for _ in range(NQ)]
        ctxT8 = _T(ctxT_p, [P, NF, SQ], F8, "ctxT8")

        # ================= phase 1: LN1, y1T8, QKV, attention =================
        with (
            tc.tile_pool(name=pfx + "y1T8", bufs=1) as y1T8_p,
            tc.tile_pool(name=pfx + "qkT", bufs=1) as qkT_p,
            tc.tile_pool(name=pfx + "wqk", bufs=NJ) as wqk_p,
            tc.tile_pool(name=pfx + "wv", bufs=NJ) as wv_p,
            tc.tile_pool(name=pfx + "Vp", bufs=NT) as V_p,
            tc.tile_pool(name=pfx + "mk", bufs=NT // 2) as mk_p,
            tc.tile_pool(name=pfx + "es", bufs=10) as e_p,
            tc.tile_pool(name=pfx + "xs", bufs=4) as xs_p,
            tc.tile_pool(name=pfx + "y1", bufs=3) as y1_p,
        ):
            y1T8 = _T(y1T8_p, [P, NF, S], F8, "y1T8")
            qT = _T(qkT_p, [P, NF, SQ], BF16, "qT")
            kT = _T(qkT_p, [P, NF, S], BF16, "kT")

            # SP DMA queue order: x0-3, wq, xk4-7, wk, m, wv, wo, w1, w2.
            # Tiles 4-7 only feed LN1 -> fp8, so the host ships them bf16.
            xt_all = []
            for tt in range(NT):
                xt_all.append(xres[tt] if tt < NQ
                              else _T(xs_p, [P, D], BF16, "xs"))
            for tt in range(NQ):
                nc.sync.dma_start(out=xt_all[tt][:], in_=x3[tt])
            wq_t = [_T(wqk_p, [P, 2, D], F8, "wq") for _ in range(NJ)]
            for j in range(NJ):
                nc.sync.dma_start(out=wq_t[j][:], in_=t["wq"][j])
            for tt in range(NQ, NT):
                nc.sync.dma_start(out=xt_all[tt][:], in_=xk3[tt - NQ])
            wk_t = [_T(wqk_p, [P, 2, D], F8, "wk") for _ in range(NJ)]
            for j in range(NJ):
                nc.sync.dma_start(out=wk_t[j][:], in_=t["wk"][j])
            m2 = [_T(mk_p, [P, 2 * SQ], mybir.dt.uint8, "m2") for _ in range(NT // 2)]
            for kp in range(NT // 2):
                nc.sync.dma_start(out=m2[kp][:], in_=t["maskT2"][kp])
            wv_t = [_T(wv_p, [P, 2, D], F8, "wv") for _ in range(NJ)]
            for j in range(NJ):
                nc.sync.dma_start(out=wv_t[j][:], in_=t["wv"][j])

            def emit_ln1_stats(tt):
                return _ln_stats(nc, small_p, xt_all[tt][:], epst)

            def emit_ln1_apply(tt, mvrs, ps_tp):
                xt = xt_all[tt]
                mv, rs = mvrs
                yt = _T(y1_p, [P, D], BF16, "y1")
                nc.gpsimd.tensor_scalar(out=yt[:], in0=xt[:],
                                        scalar1=mv[:, 0:1], scalar2=rs[:],
                                        op0=ALU.subtract, op1=ALU.mult)
                for fc in range(NF):
                    pt = _T(ps_tp, [P, P], BF16, "pt")
                    nc.tensor.transpose(pt[:], yt[:, fc * P:(fc + 1) * P],
                                        ident[:])
                    dst = y1T8[:, fc, tt * P:(tt + 1) * P]
                    if fc < 6:
                        nc.scalar.copy(out=dst, in_=pt[:])
                    else:
                        nc.vector.tensor_copy(out=dst, in_=pt[:])

            with (
                tc.tile_pool(name=pfx + "ps_s2", bufs=2,
                             space=bass.MemorySpace.PSUM) as ps_s2,
                tc.tile_pool(name=pfx + "ps_qkv", bufs=1,
                             space=bass.MemorySpace.PSUM) as ps_qkv,
            ):
                def kT_chunk(fo, th):
                    tsl = slice(th * SQ, (th + 1) * SQ)
                    ps = _T(ps_qkv, [P, 2 * SQ], F32, "pqk")
                    half = ps[:, 0:SQ] if th == 0 else ps[:, SQ:2 * SQ]
                    for j in range(NJ):
                        nc.tensor.matmul(half,
                                         wk_t[j][:, :, fo * P:(fo + 1) * P],
                                         y1T8[:, 2 * j:2 * j + 2, tsl],
                                         start=(j == 0),
                                         stop=(not FL["bk"] and j == NJ - 1),
                                         perf_mode=DR)
                    if FL["bk"]:
                        nc.tensor.matmul(half,
                                         brow["bk"][:, fo * P:(fo + 1) * P],
                                         ones[:], start=False, stop=True)
                    nc.vector.tensor_copy(out=kT[:, fo, tsl], in_=half)

                def score2(h, kp):
                    """scores^T -> exp for one (head, kt-pair): E [128,1024]
                    covering key tiles 2kp, 2kp+1."""
                    fc, po = h // 2, (h % 2) * d
                    ps2 = _T(ps_s2, [P, 2 * SQ], F32, "ps2")
                    for sub in range(2):
                        kt = 2 * kp + sub
                        nc.tensor.matmul(ps2[:, sub * SQ:(sub + 1) * SQ],
                                         kT[po:po + d, fc, kt * P:(kt + 1) * P],
                                         qT[po:po + d, fc, :],
                                         start=True, stop=True)
                    e = _T(e_p, [P, 2 * SQ], BF16, "e")
                    nc.scalar.activation(out=e[:], in_=ps2[:], func=ACTF.Exp,
                                         scale=1.0 / (SCQ * SCK))
                    nc.gpsimd.tensor_mul(e[:], e[:], m2[kp][:])
                    return e

                V = [_T(V_p, [P, H, 2 * d], BF16, "V") for _ in range(NT)]

                def V_mm(kt):
                    # V token-major, heads interleaved d v-cols then d 16-cols
                    nc.gpsimd.memset(V[kt][:, :, d:], SCV)
                    ps = _T(ps_s2, [P, 2 * SQ], F32, "ps2")
                    for fh in range(2):
                        half = ps[:, fh * SQ:(fh + 1) * SQ]
                        for j in range(NJ):
                            nc.tensor.matmul(half,
                                             y1T8[:, 2 * j:2 * j + 2,
                                                  kt * P:(kt + 1) * P],
                                             wv_t[j][:, :, fh * SQ:(fh + 1) * SQ],
                                             start=(j == 0),
                                             stop=(not FL["bv"] and j == NJ - 1),
                                             perf_mode=DR)
                        if FL["bv"]:
                            nc.tensor.matmul(half, ones[:, 0:P],
                                             brow["bv"][:, fh * SQ:(fh + 1) * SQ],
                                             start=False, stop=True)
                    nc.vector.tensor_copy(
                        out=V[kt][:, :, 0:d],
                        in_=ps[:].rearrange("p (a b) -> p a b", b=d))

                e01 = {0: [], 1: []}
                with tc.tile_pool(name=pfx + "ps_tp", bufs=2,
                                  space=bass.MemorySpace.PSUM) as ps_tp:
                    for tt in range(NQ):
                        emit_ln1_apply(tt, emit_ln1_stats(tt), ps_tp)

                    # q^T: fp8 DoubleRow, fo pairs share a PSUM buf (own 512 q)
                    for fe in range(0, NF, 2):
                        ps = _T(ps_qkv, [P, 2 * SQ], F32, "pqk")
                        for sub in range(2):
                            fo = fe + sub
                            half = ps[:, sub * SQ:(sub + 1) * SQ]
                            for j in range(NJ):
                                nc.tensor.matmul(half,
                                                 wq_t[j][:, :, fo * P:(fo + 1) * P],
                                                 y1T8[:, 2 * j:2 * j + 2, 0:SQ],
                                                 start=(j == 0),
                                                 stop=(not FL["bq"] and j == NJ - 1),
                                                 perf_mode=DR)
                            if FL["bq"]:
                                nc.tensor.matmul(half,
                                                 brow["bq"][:, fo * P:(fo + 1) * P],
                                                 ones[:], start=False, stop=True)
                        nc.vector.tensor_copy(
                            out=qT[:, fe:fe + 2, :],
                            in_=ps[:].rearrange("p (a b) -> p a b", b=SQ))

                    kT_chunk(0, 0)
                    kT_chunk(1, 0)
                    # stats (and their ACT sqrts) for tiles 4-7 queue before
                    # the first exp so the Sqrt/Exp tables load exactly once
                    mvrs = {tt: emit_ln1_stats(tt) for tt in range(NQ, NT)}
                    # pair-0 scores for the first two kt-pairs start while the
                    # second half of the sequence is still loading/normalizing
                    for kp in (0, 1):
                        for h in (0, 1):
                            e01[h].append(score2(h, kp))
                    emit_ln1_apply(NQ, mvrs[NQ], ps_tp)
                    emit_ln1_apply(NQ + 1, mvrs[NQ + 1], ps_tp)
                    V_mm(0)
                    emit_ln1_apply(NQ + 2, mvrs[NQ + 2], ps_tp)
                    V_mm(1)
                    emit_ln1_apply(NQ + 3, mvrs[NQ + 3], ps_tp)
                    V_mm(2)
                    V_mm(3)

                kT_chunk(0, 1)
                kT_chunk(1, 1)
                for kt in range(4, NT):
                    V_mm(kt)
                for kp in (2, 3):
                    for h in (0, 1):
                        e01[h].append(score2(h, kp))
                kT_chunk(2, 0)
                kT_chunk(2, 1)

                with tc.tile_pool(name=pfx + "ps_ctx", bufs=2,
                                  space=bass.MemorySpace.PSUM) as ps_ctx:
                    def ctx_mm(pcs, h, kp, e):
                        for sub in range(2):
                            kt = 2 * kp + sub
                            nc.tensor.matmul(pcs[:], V[kt][:, h, :],
                                             e[:, sub * SQ:(sub + 1) * SQ],
                                             start=(kt == 0), stop=(kt == NT - 1),
                                             skip_group_check=True)

                    def ctx_norm(h, pcs):
                        fco, po = h // 2, (h % 2) * d
                        rb = _T(small_p, [d, SQ], F32, "rb")
                        nc.vector.reciprocal(rb[:], pcs[d:2 * d, :])
                        nc.vector.tensor_mul(ctxT8[po:po + 64, fco, :],
                                             pcs[0:d, :], rb[:])

                    pcs0 = {h: _T(ps_ctx, [P, SQ], F32, "pc") for h in (0, 1)}
                    for kp in range(NT // 2):
                        for h in (0, 1):
                            ctx_mm(pcs0[h], h, kp, e01[h][kp])
                    for h in (0, 1):
                        ctx_norm(h, pcs0[h])
                    del e01, pcs0

                    LAG = 2
                    for fc in range(1, NF):
                        pair = (2 * fc, 2 * fc + 1)
                        es = {h: [] for h in pair}
                        pcs = {h: _T(ps_ctx, [P, SQ], F32, "pc") for h in pair}
                        for kp in range(NT // 2):
                            for h in pair:
                                es[h].append(score2(h, kp))
                            if kp == 0 and fc + 2 < NF:
                                kT_chunk(fc + 2, 0)
                                kT_chunk(fc + 2, 1)
                            if kp >= LAG:
                                for h in pair:
                                    ctx_mm(pcs[h], h, kp - LAG, es[h][kp - LAG])
                        for kp in range(NT // 2 - LAG, NT // 2):
                            for h in pair:
                                ctx_mm(pcs[h], h, kp, es[h][kp])
                        for h in pair:
                            ctx_norm(h, pcs[h])

        # ================= phase 2: Wo, LN2, FFN =================
        with (
            tc.tile_pool(name=pfx + "wo", bufs=2 * NJ) as wo_p,
            tc.tile_pool(name=pfx + "x2", bufs=NQ) as x2_p,
            tc.tile_pool(name=pfx + "y2", bufs=2) as y2_p,
            tc.tile_pool(name=pfx + "y2T", bufs=1) as y2T_p,
            tc.tile_pool(name=pfx + "x2t", bufs=2) as x2t_p,
        ):
            wo_t = [_T(wo_p, [P, 2, SQ], F8, "woW") for _ in range(2 * NJ)]
            for i in range(2 * NJ):
                nc.sync.dma_start(out=wo_t[i][:], in_=t["wo"][i])
            w1_t = [_T(w1_p, [P, D], BF16, "w1s") for _ in range(NF1)]
            for f1 in range(NF1):
                nc.sync.dma_start(out=w1_t[f1][:], in_=t["w1"][f1])
            w2_t = [_T(wh_p, [P, SQ], BF16, "wh") for _ in range(2 * NF1)]
            for i in range(2 * NF1):
                nc.sync.dma_start(out=w2_t[i][:], in_=t["w2"][i])

            x2 = [_T(x2_p, [P, D], F32, "x2") for _ in range(NQ)]
            y2T = _T(y2T_p, [P, NF, SQ], BF16, "y2T")

            with (
                tc.tile_pool(name=pfx + "ps_wo", bufs=2,
                             space=bass.MemorySpace.PSUM) as ps_wo,
                tc.tile_pool(name=pfx + "ps_tp2", bufs=2,
                             space=bass.MemorySpace.PSUM) as ps_tp2,
            ):
                def emit_wo(qt, ps):
                    for dh in range(2):
                        half = ps[:, dh * SQ:(dh + 1) * SQ]
                        for j in range(NJ):
                            nc.tensor.matmul(
                                half,
                                ctxT8[:, 2 * j:2 * j + 2, qt * P:(qt + 1) * P],
                                wo_t[dh * NJ + j][:],
                                start=(j == 0), stop=(j == NJ - 1),
                                perf_mode=DR)

                def emit_ln2(qt, ps):
                    xt2 = _T(x2t_p, [P, D], F32, "x2t")
                    nc.scalar.activation(out=xt2[:], in_=ps[:], func=ACTF.Copy,
                                         scale=1.0 / SCO)
                    nc.vector.tensor_add(x2[qt][:], xt2[:], xres[qt][:])
                    if FL["bo"]:
                        nc.vector.tensor_add(x2[qt][:], x2[qt][:], bo_b[:])
                    yt = _T(y2_p, [P, D], BF16, "y2")
                    mv, rs = _ln_stats(nc, small_p, x2[qt][:], epst)
                    nc.vector.tensor_scalar(out=yt[:], in0=x2[qt][:],
                                            scalar1=mv[:, 0:1], scalar2=rs[:],
                                            op0=ALU.subtract, op1=ALU.mult)
                    return yt

                def emit_y2T(qt, yt):
                    for fc in range(NF):
                        pt = _T(ps_tp2, [P, P], BF16, "pt2")
                        nc.tensor.transpose(pt[:], yt[:, fc * P:(fc + 1) * P],
                                            ident[:])
                        nc.scalar.copy(out=y2T[:, fc, qt * P:(qt + 1) * P],
                                       in_=pt[:])

                wops = [_T(ps_wo, [P, D], F32, "pwo") for _ in range(2)]
                emit_wo(0, wops[0])
                emit_wo(1, wops[1])
                y0 = emit_ln2(0, wops[0])
                y1_ = emit_ln2(1, wops[1])
                wops2 = [_T(ps_wo, [P, D], F32, "pwo") for _ in range(2)]
                emit_wo(2, wops2[0])
                emit_y2T(0, y0)
                emit_wo(3, wops2[1])
                emit_y2T(1, y1_)
                y2_ = emit_ln2(2, wops2[0])
                y3 = emit_ln2(3, wops2[1])
                emit_y2T(2, y2_)
                emit_y2T(3, y3)

            # FFN1 (bf16): h^T = gelu(W1' y2 + b1'), f1 pairs share a PSUM buf
            with (
                tc.tile_pool(name=pfx + "hT", bufs=1) as hT_p,
                tc.tile_pool(name=pfx + "xo", bufs=NQ) as xo_p,
                tc.tile_pool(name=pfx + "ps_f1", bufs=2,
                             space=bass.MemorySpace.PSUM) as ps_f1,
                tc.tile_pool(name=pfx + "ps_4", bufs=4,
                             space=bass.MemorySpace.PSUM) as ps_4,
            ):
                hT = _T(hT_p, [P, NF1, SQ], BF16, "hT")
                for fe in range(0, NF1, 2):
                    ps = _T(ps_f1, [P, 2 * SQ], F32, "pf1")
                    for sub in range(2):
                        f1 = fe + sub
                        half = ps[:, sub * SQ:(sub + 1) * SQ]
                        for fi in range(NF):
                            nc.tensor.matmul(half,
                                             w1_t[f1][:, fi * P:(fi + 1) * P],
                                             y2T[:, fi, :],
                                             start=(fi == 0),
                                             stop=(not FL["b1"] and fi == NF - 1))
                        if FL["b1"]:
                            nc.tensor.matmul(half,
                                             brow["b1"][:, f1 * P:(f1 + 1) * P],
                                             ones[:], start=False, stop=True)
                    nc.scalar.activation(out=hT[:, fe:fe + 2, :].rearrange(
                        "p a b -> p (a b)"), in_=ps[:], func=ACTF.Gelu)

                # FFN2 + residual: out = x2 + h @ W2 + b2
                xout = [_T(xo_p, [P, D], F32, "xo") for _ in range(NQ)]
                for dh in range(2):
                    dsl = slice(dh * SQ, (dh + 1) * SQ)
                    ps4 = [_T(ps_4, [P, SQ], F32, "p4") for _ in range(NQ)]
                    for f1 in range(NF1):
                        for qt in range(NQ):
                            nc.tensor.matmul(ps4[qt][:],
                                             hT[:, f1, qt * P:(qt + 1) * P],
                                             w2_t[dh * NF1 + f1][:],
                                             start=(f1 == 0),
                                             stop=(not FL["b2"] and f1 == NF1 - 1))
                    for qt in range(NQ):
                        if FL["b2"]:
                            nc.tensor.matmul(ps4[qt][:], ones[:, 0:P],
                                             brow["b2"][:, dsl],
                                             start=False, stop=True)
                        nc.vector.tensor_add(xout[qt][:, dsl], ps4[qt][:],
                                             x2[qt][:, dsl])
                        nc.sync.dma_start(out=out3[qt][:, dsl],
                                          in_=xout[qt][:, dsl])


_NC = {}
_ALL_FLAGS = ("bq", "bk", "bv", "bo", "b1", "b2")


def _get_nc(flags=None, reps=1):
    if flags is None:
        flags = {k: True for k in _ALL_FLAGS}
    key = (tuple(sorted(flags.items())), reps)
    if key not in _NC:
        _NC[key] = _build_program(dict(flags), reps=reps)
    return _NC[key]


def _q8(w, scale):
    return np.clip(w * scale, -224.0, 224.0).astype(E4)


def _prep_inputs(inputs):
    """Host-side folding + per-core shard maps."""
    x = np.asarray(inputs["x"], np.float32)
    mask = np.asarray(inputs["mask"], np.float32)
    g1 = np.asarray(inputs["ln1_g"], np.float32)
    b1n = np.asarray(inputs["ln1_b"], np.float32)
    g2 = np.asarray(inputs["ln2_g"], np.float32)
    b2n = np.asarray(inputs["ln2_b"], np.float32)
    Wq = np.asarray(inputs["Wq"], np.float32); bq = np.asarray(inputs["bq"], np.float32)
    Wk = np.asarray(inputs["Wk"], np.float32); bk = np.asarray(inputs["bk"], np.float32)
    Wv = np.asarray(inputs["Wv"], np.float32); bv = np.asarray(inputs["bv"], np.float32)
    Wo = np.asarray(inputs["Wo"], np.float32); bo = np.asarray(inputs["bo"], np.float32)
    W1 = np.asarray(inputs["W1"], np.float32); b1 = np.asarray(inputs["b1"], np.float32)
    W2 = np.asarray(inputs["W2"], np.float32); b2 = np.asarray(inputs["b2"], np.float32)

    scale = d ** -0.5
    # fold LN gains/biases (and q scale) into the projection weights
    Wq_e = (g1[:, None] * Wq) * scale
    bq_e = (b1n @ Wq + bq) * scale
    Wk_e = g1[:, None] * Wk
    bk_e = b1n @ Wk + bk
    Wv_e = g1[:, None] * Wv
    bv_e = b1n @ Wv + bv
    W1_e = g2[:, None] * W1
    b1_e = b2n @ W1 + b1

    def tile_dr(W, sc):
        # [Din, Dout] -> [NJ(j), P, 2(i), Dout] fp8, f_in = (2j+i)*128 + p
        return np.ascontiguousarray(
            _q8(W, sc).reshape(NJ, 2, P, D).transpose(0, 2, 1, 3))

    wq_h = tile_dr(Wq_e, SCQ)
    wk_h = tile_dr(Wk_e, SCK)
    wv_h = tile_dr(Wv_e, SCV)
    # Wo -> [2(dh)*NJ(j), P, 2(i), SQ] fp8
    wo_h = np.ascontiguousarray(
        _q8(Wo, SCO).reshape(NJ, 2, P, 2, SQ).transpose(3, 0, 2, 1, 4)
        .reshape(2 * NJ, P, 2, SQ))
    # W1 -> [NF1(f1), P(f_in within chunk), NF(fi)*P]
    w1_h = np.ascontiguousarray(
        W1_e.reshape(NF, P, NF1, P).transpose(2, 1, 0, 3).reshape(NF1, P, D)
    ).astype(BF)
    # W2 -> [2*NF1 (dh,f1), P, SQ]
    w2_h = np.ascontiguousarray(
        W2.reshape(NF1, P, 2, SQ).transpose(2, 0, 1, 3).reshape(2 * NF1, P, SQ)
    ).astype(BF)

    flags = {
        "bq": bool(np.any(bq_e)), "bk": bool(np.any(bk_e)),
        "bv": bool(np.any(bv_e)), "bo": bool(np.any(bo)),
        "b1": bool(np.any(b1_e)), "b2": bool(np.any(b2)),
    }
    shared = {
        "ident": np.eye(P, dtype=BF),
        "wq": wq_h, "wk": wk_h, "wv": wv_h, "wo": wo_h, "w1": w1_h, "w2": w2_h,
        "bq": (bq_e * SCQ).reshape(1, D).astype(BF),
        "bk": (bk_e * SCK).reshape(1, D).astype(BF),
        "bv": (bv_e * SCV).reshape(1, D).astype(BF),
        "bo": bo.reshape(1, D).astype(np.float32),
        "b1": b1_e.reshape(1, F).astype(BF),
        "b2": b2.reshape(1, D).astype(BF),
    }

    in_maps = []
    for c in range(8):
        b, hf = c // 2, c % 2
        x_rot = np.ascontiguousarray(np.roll(x[b], -SQ * hf, axis=0))
        mq = mask[b, 0, SQ * hf:SQ * (hf + 1), :]          # [512 q, 1024 k]
        mT = np.roll(mq.T, -SQ * hf, axis=0)               # [1024 k, 512 q]
        # kt-pair layout: [4, 128, 1024] with pair tiles side by side
        mT2 = np.ascontiguousarray(
            mT.reshape(NT // 2, 2, P, SQ).transpose(0, 2, 1, 3)
            .reshape(NT // 2, P, 2 * SQ)).astype(np.uint8)
        m = dict(shared)
        m["x"] = np.ascontiguousarray(x_rot[:SQ])
        m["xk"] = np.ascontiguousarray(x_rot[SQ:]).astype(BF)
        m["maskT2"] = mT2
        in_maps.append(m)
    return in_maps, flags


def run(inputs, trace=False, **kw):
    in_maps, flags = _prep_inputs(inputs)
    nc = _get_nc(flags)
    res = run_bass_kernel_spmd(nc, in_maps, core_ids=list(range(8)),
                               trace=trace, **kw)
    out = np.empty((B, S, D), np.float32)
    for c in range(8):
        b, hf = c // 2, c % 2
        out[b, SQ * hf:SQ * (hf + 1), :] = res.results[c]["out"]
    return out, res


def kernel(**inputs) -> np.ndarray:
    out, _ = run(inputs, trace=False)
    return out
